# revision 16
# baseline (speedup 1.0000x reference)
"""Masked attention on 8 TRN2 NeuronCores.

Full-input contract: kernel(**inputs) takes the complete Q/K/V/mask/dk and
returns the full [32, 2048, 64] output. Internally shards batch 32 -> 4 per
core (data parallel, no communication).

Per-core kernel (4 batches of S=2048, D=64). Scores computed TRANSPOSED
(S^T[k,q] = K @ Q^T) so exp's output P^T is already in the layout the P@V
matmul consumes. v3 design notes (calibrated to this part's PE clock, which
duty-cycles ~1.28/0.68 GHz on a fixed ~102us firmware period):
  - every matmul operand is bf16/fp8 (fp32/fp32r matmuls leave the clock
    governor cold)
  - QK^T: bf16, 2-way PE row tiling; a chunk PAIR shares one PSUM tile
    [128, 2, 512] so the two row-tiled matmuls become adjacent instructions
    with identical dependencies and can co-issue on opposite 64-row halves
  - mask: split between engines to balance load.
      PE pairs (PE_MASK_PAIRS): additive fp8 {0,-240} mask accumulated onto
        scores by 4 concurrent 64x64 quadrant identity-matmuls (odd chunk
        stored half-rolled so off-diagonal quadrants route its rows); exp
        then underflows masked entries to ~1e-13.
      DVE pairs (the rest): bf16 {1,0} keep-mask multiplied into exp's
        output (2x-mode tensor_tensor, one op per pair-tile).
  - exp on ScalarE (scale=1/8 folded in), bf16 out, one op per [128,2,512]
    pair-tile
  - P@V transposed: V chunk (with a ones column computing the softmax row
    sum) is the stationary operand, P^T the moving one -> outT[66, q]
    accumulates in PSUM over k-chunks
  - epilogue: copy outT to SBUF (bf16), transpose each 128-q block back via
    a regular matmul against a 66-wide identity, then DVE reciprocal of the
    row-sum column + per-partition scale -> natural [q, d] fp32 output
"""

import sys

import numpy as np

for _p in ("/opt/trn_rl_repo", "/root/.axon_site/_ro/trn_rl_repo"):
    if _p not in sys.path:
        sys.path.append(_p)

import ml_dtypes

import concourse.bacc as bacc
import concourse.bass as bass
import concourse.mybir as mybir
from concourse.bass_utils import run_bass_kernel_spmd
from concourse.tile import TileContext

N_CORES = 8
B, S, D = 32, 2048, 64
BPC = B // N_CORES  # batches per core
NK = S // 128  # 16 k-chunks
NP = NK // 2  # 8 chunk pairs
NH = 2  # q halves
QH = S // NH  # 1024
VW = 66  # V free width: 64 d + 1 ones + 1 pad
NEG = -240.0  # additive mask value; max normal of TRN fp8e4

# which chunk pairs fold their mask on the PE (fp8 quadrant matmuls) vs
# multiplying on the DVE (bf16). Tune to balance PE vs DVE vs DMA load.
PE_MASK_PAIRS = (7,)
DVE_MASK_PAIRS = tuple(p for p in range(NP) if p not in PE_MASK_PAIRS)
NPEP = len(PE_MASK_PAIRS)
NDVP = len(DVE_MASK_PAIRS)
# pairs whose exp runs as a Schraudolph fast-exp on the DVE (one
# tensor_scalar producing bf16 bits via an int16 affine) instead of on
# ScalarE. Must be PE-masked pairs (the additive -240 fold makes masked
# entries underflow to ~2^-44 in the bit-trick too).
SCHRAUD_PAIRS = ()
# i16 = s * (0.125 * log2(e) * 128) + (16256 - 5.5); bitcast int16->bf16
SCH_A = 23.083120654223414
SCH_B = 16250.5

F32 = mybir.dt.float32
BF16 = mybir.dt.bfloat16
FP8 = mybir.dt.float8e4
EXP = mybir.ActivationFunctionType.Exp

_CACHED_NC = None


def build_nc():
    global _CACHED_NC
    if _CACHED_NC is not None:
        return _CACHED_NC
    nc = bacc.Bacc("TRN2", target_bir_lowering=False)
    QT = nc.dram_tensor("qt", [BPC, 128, S], BF16, kind="ExternalInput")
    KT = nc.dram_tensor("kt", [BPC, 128, S], BF16, kind="ExternalInput")
    V2 = nc.dram_tensor("v2", [BPC, 128, NK * VW], BF16, kind="ExternalInput")
    # additive mask for PE pairs: {0, -240} fp8, [b, h, p, pep, c01, q];
    # the c01=1 chunk is stored rolled by 64 partitions
    M8 = nc.dram_tensor(
        "m8", [BPC, NH, 128, max(NPEP, 1), 2, QH], FP8, kind="ExternalInput"
    )
    # keep-mask for DVE pairs: {1, 0} bf16, [b, h, p, dvp, c01, j, q]
    NM = nc.dram_tensor(
        "nm", [BPC, NH, 128, max(NDVP, 1), 2, 2, 512], BF16, kind="ExternalInput"
    )
    # identity duplicated into both partition halves: id2[p, c] = (p % 64 == c)
    ID2 = nc.dram_tensor("id2", [128, 64], FP8, kind="ExternalInput")
    ID66 = nc.dram_tensor("id66", [VW, VW], BF16, kind="ExternalInput")
    OUT = nc.dram_tensor("out", [BPC, NH, 8, 128, D], F32, kind="ExternalOutput")

    with TileContext(nc) as tc:
        with (
            tc.tile_pool(name="cn", bufs=1) as const_pool,
            tc.tile_pool(name="qk", bufs=2) as qk_pool,
            tc.tile_pool(name="vp", bufs=2) as v_pool,
            tc.tile_pool(name="m8", bufs=3) as m_pool,
            tc.tile_pool(name="pt", bufs=6) as pt_pool,
            tc.tile_pool(name="os", bufs=2) as ots_pool,
            tc.tile_pool(name="sc", bufs=3, space="PSUM") as sc_pool,
            tc.tile_pool(name="pv", bufs=1, space="PSUM") as pv_pool,
            tc.tile_pool(name="ou", bufs=2) as out_pool,
            tc.tile_pool(name="mi", bufs=4) as misc_pool,
        ):
            id2 = const_pool.tile([128, 64], FP8, tag="id2")
            nc.sync.dma_start(out=id2, in_=ID2[:, :])
            id66 = const_pool.tile([VW, VW], BF16, tag="id66")
            nc.sync.dma_start(out=id66, in_=ID66[:, :])
            for b in range(BPC):
                qt = qk_pool.tile([128, S], BF16, tag="qt")
                kt = qk_pool.tile([128, S], BF16, tag="kt")
                v2 = v_pool.tile([128, NK * VW], BF16, tag="v2")
                nc.sync.dma_start(out=qt, in_=QT[b])
                nc.sync.dma_start(out=kt, in_=KT[b])
                nc.sync.dma_start(out=v2, in_=V2[b])
                for h in range(NH):
                    m8 = m_pool.tile([128, max(NPEP, 1), 2, QH], FP8, tag="m8")
                    nc.sync.dma_start(out=m8, in_=M8[b, h])
                    nm = m_pool.tile([128, max(NDVP, 1), 2, 2, 512], BF16, tag="nm")
                    nc.sync.dma_start(out=nm, in_=NM[b, h])
                    # transposed PV accumulator: [d'=66, q=1024], 2 banks
                    outT = pv_pool.tile([VW, QH], F32, tag="pv")
                    for p in range(NP):
                        c0, c1 = 2 * p, 2 * p + 1
                        pe_mask = p in PE_MASK_PAIRS
                        pep = PE_MASK_PAIRS.index(p) if pe_mask else 0
                        dvp = DVE_MASK_PAIRS.index(p) if not pe_mask else 0
                        schraud = p in SCHRAUD_PAIRS
                        # one pt tile per pair [p, j, c01, q] so the DVE
                        # mask multiply is a single 2048-wide op
                        ptp = pt_pool.tile(
                            [128, 2, 2, 512],
                            mybir.dt.int16 if schraud else BF16,
                            tag="pt",
                        )
                        for j in range(2):
                            q0 = h * QH + j * 512
                            sc = sc_pool.tile([128, 2, 512], F32, tag="sc")
                            # QK on alternating 64-row halves; adjacent
                            # instructions + shared tile -> co-issue
                            nc.tensor.matmul(
                                sc[:, 0, :],
                                kt[0:64, c0 * 128 : (c0 + 1) * 128],
                                qt[0:64, q0 : q0 + 512],
                                start=True,
                                stop=not pe_mask,
                            )
                            nc.tensor.matmul(
                                sc[:, 1, :],
                                kt[64:128, c1 * 128 : (c1 + 1) * 128],
                                qt[64:128, q0 : q0 + 512],
                                start=True,
                                stop=not pe_mask,
                            )
                            if pe_mask:
                                jl = slice(j * 512, (j + 1) * 512)
                                # 4 concurrent 64x64 quadrant matmuls add the
                                # fp8 mask onto the scores in PSUM
                                nc.tensor.matmul(
                                    sc[0:64, 0, :],
                                    id2[0:64, :],
                                    m8[0:64, pep, 0, jl],
                                    start=False,
                                    stop=True,
                                )
                                nc.tensor.matmul(
                                    sc[64:128, 0, :],
                                    id2[64:128, :],
                                    m8[64:128, pep, 0, jl],
                                    start=False,
                                    stop=True,
                                )
                                nc.tensor.matmul(
                                    sc[64:128, 1, :],
                                    id2[0:64, :],
                                    m8[0:64, pep, 1, jl],
                                    start=False,
                                    stop=True,
                                )
                                nc.tensor.matmul(
                                    sc[0:64, 1, :],
                                    id2[64:128, :],
                                    m8[64:128, pep, 1, jl],
                                    start=False,
                                    stop=True,
                                )
                            if schraud:
                                nc.vector.tensor_scalar(
                                    ptp[:, j],
                                    sc,
                                    SCH_A,
                                    SCH_B,
                                    mybir.AluOpType.mult,
                                    mybir.AluOpType.add,
                                )
                            else:
                                nc.scalar.activation(ptp[:, j], sc, EXP, scale=0.125)
                        if not pe_mask:
                            # nm is [p, c01, j, q]; view as [p, j, c01, q]
                            nc.vector.tensor_mul(
                                ptp,
                                ptp,
                                nm[:, dvp].rearrange("p c j q -> p j c q"),
                            )
                        ptv = ptp.bitcast(BF16) if schraud else ptp
                        for ci, c in enumerate((c0, c1)):
                            for j in range(2):
                                jl = slice(j * 512, (j + 1) * 512)
                                nc.tensor.matmul(
                                    outT[:, jl],
                                    v2[:, c * VW : (c + 1) * VW],
                                    ptv[:, j, ci, :],
                                    start=(c == 0),
                                    stop=(c == NK - 1),
                                )
                    # epilogue: outT -> SBUF bf16 -> matmul-transpose each
                    # 128-q block -> reciprocal of row-sum col -> scale
                    ots = ots_pool.tile([VW, QH], BF16, tag="ots")
                    nc.scalar.copy(ots, outT)
                    ot_all = out_pool.tile([128, 8, D], F32, tag="ot")
                    for r in range(2):
                        trans = pv_pool.tile([128, 4, 128], F32, tag="pv", name=f"tr{r}")
                        for qq in range(4):
                            qb = r * 4 + qq
                            nc.tensor.matmul(
                                trans[:, qq, 0:VW],
                                ots[:, qb * 128 : (qb + 1) * 128],
                                id66,
                                start=True,
                                stop=True,
                            )
                        rec = misc_pool.tile([128, 4], F32, tag="rec")
                        nc.vector.reciprocal(rec, trans[:, :, 64])
                        for qq in range(4):
                            nc.vector.tensor_scalar_mul(
                                ot_all[:, r * 4 + qq, :],
                                trans[:, qq, 0:64],
                                rec[:, qq : qq + 1],
                            )
                    nc.sync.dma_start(
                        out=OUT[b, h].rearrange("a p d -> p a d"), in_=ot_all
                    )
    nc.compile()
    _CACHED_NC = nc
    return nc


def prep_inputs(Q, K, V, mask):
    """Host-side layout prep (transposes, duplication for row tiling, bf16)."""
    Q = np.ascontiguousarray(np.asarray(Q, dtype=np.float32))
    K = np.ascontiguousarray(np.asarray(K, dtype=np.float32))
    V = np.ascontiguousarray(np.asarray(V, dtype=np.float32))
    mask = np.asarray(mask)
    QT1 = Q.transpose(0, 2, 1)  # [B, D, S]
    KT1 = K.transpose(0, 2, 1)
    QT = np.ascontiguousarray(
        np.concatenate([QT1, QT1], axis=1).astype(ml_dtypes.bfloat16)
    )  # [B, 128, S]
    KT = np.ascontiguousarray(
        np.concatenate([KT1, KT1], axis=1).astype(ml_dtypes.bfloat16)
    )
    # V with ones column (row-sum trick) + pad, interleaved so each SBUF
    # partition's 16 chunks are contiguous in DRAM: [B, 128, 16*VW]
    V66 = np.zeros((B, S, VW), dtype=ml_dtypes.bfloat16)
    V66[:, :, :64] = V.astype(ml_dtypes.bfloat16)
    V66[:, :, 64] = 1.0
    V2 = np.ascontiguousarray(
        V66.reshape(B, NK, 128, VW).transpose(0, 2, 1, 3).reshape(B, 128, NK * VW)
    )
    mt = mask.astype(bool).transpose(0, 2, 1)  # [B, k, q]
    mt = mt.reshape(B, NP, 2, 128, NH, QH)  # [b, pair, c01, p, h, q]
    # PE pairs: additive fp8; odd chunk rolled by 64 partitions so the
    # off-diagonal quadrant matmuls route its rows to the right partitions
    pe = mt[:, list(PE_MASK_PAIRS)]  # [b, pep, c01, p, h, q]
    pe = np.stack([pe[:, :, 0], np.roll(pe[:, :, 1], -64, axis=2)], axis=2)
    M8 = np.ascontiguousarray(
        (pe.astype(np.float32) * NEG)
        .transpose(0, 4, 3, 1, 2, 5)  # [b, h, p, pep, c01, q]
        .astype(ml_dtypes.float8_e4m3)
    )
    # DVE pairs: keep-mask bf16 [b, h, p, dvp, c01, j, q512]
    dv = ~mt[:, list(DVE_MASK_PAIRS)]  # [b, dvp, c01, p, h, q]
    dv = dv.reshape(B, NDVP, 2, 128, NH, 2, 512)
    NM = np.ascontiguousarray(
        dv.transpose(0, 4, 3, 1, 2, 5, 6).astype(ml_dtypes.bfloat16)
    )  # [b, h, p, dvp, c01, j, q]
    id2 = np.zeros((128, 64), dtype=ml_dtypes.float8_e4m3)
    id2[np.arange(128), np.arange(128) % 64] = 1.0
    id66 = np.eye(VW, dtype=ml_dtypes.bfloat16)
    return QT, KT, V2, M8, NM, id2, id66


def make_in_maps(Q, K, V, mask):
    QT, KT, V2, M8, NM, id2, id66 = prep_inputs(Q, K, V, mask)
    in_maps = []
    for i in range(N_CORES):
        sl = slice(i * BPC, (i + 1) * BPC)
        in_maps.append(
            {
                "qt": QT[sl],
                "kt": KT[sl],
                "v2": V2[sl],
                "m8": M8[sl],
                "nm": NM[sl],
                "id2": id2,
                "id66": id66,
            }
        )
    return in_maps


def kernel(Q, K, V, mask, dk, **run_kwargs):
    assert int(dk) == D
    nc = build_nc()
    in_maps = make_in_maps(Q, K, V, mask)
    res = run_bass_kernel_spmd(nc, in_maps, list(range(N_CORES)), **run_kwargs)
    out = np.concatenate(
        [res.results[i]["out"].reshape(BPC, S, D) for i in range(N_CORES)], axis=0
    )
    if run_kwargs:
        kernel.last_results = res
    return out


# revision 18
# speedup vs baseline: 1.2273x; 1.2273x over previous
"""Masked attention on 8 TRN2 NeuronCores.

Full-input contract: kernel(**inputs) takes the complete Q/K/V/mask/dk and
returns the full [32, 2048, 64] output. Internally shards batch 32 -> 4 per
core (data parallel, no communication).

Per-core kernel (4 batches of S=2048, D=64). Scores computed TRANSPOSED
(S^T[k,q] = K @ Q^T) so exp's output P^T is already in the layout the P@V
matmul consumes. v3 design notes (calibrated to this part's PE clock, which
duty-cycles ~1.28/0.68 GHz on a fixed ~102us firmware period):
  - every matmul operand is bf16/fp8 (fp32/fp32r matmuls leave the clock
    governor cold)
  - QK^T: bf16, 2-way PE row tiling; a chunk PAIR shares one PSUM tile
    [128, 2, 512] so the two row-tiled matmuls become adjacent instructions
    with identical dependencies and can co-issue on opposite 64-row halves
  - mask: split between engines to balance load.
      PE pairs (PE_MASK_PAIRS): additive fp8 {0,-240} mask accumulated onto
        scores by 4 concurrent 64x64 quadrant identity-matmuls (odd chunk
        stored half-rolled so off-diagonal quadrants route its rows); exp
        then underflows masked entries to ~1e-13.
      DVE pairs (the rest): bf16 {1,0} keep-mask multiplied into exp's
        output (2x-mode tensor_tensor, one op per pair-tile).
  - exp on ScalarE (scale=1/8 folded in), bf16 out, one op per [128,2,512]
    pair-tile
  - P@V transposed: V chunk (with a ones column computing the softmax row
    sum) is the stationary operand, P^T the moving one -> outT[66, q]
    accumulates in PSUM over k-chunks
  - epilogue: copy outT to SBUF (bf16), transpose each 128-q block back via
    a regular matmul against a 66-wide identity, then DVE reciprocal of the
    row-sum column + per-partition scale -> natural [q, d] fp32 output
"""

import sys

import numpy as np

for _p in ("/opt/trn_rl_repo", "/root/.axon_site/_ro/trn_rl_repo"):
    if _p not in sys.path:
        sys.path.append(_p)

import ml_dtypes

import concourse.bacc as bacc
import concourse.bass as bass
import concourse.mybir as mybir
from concourse.bass_utils import run_bass_kernel_spmd
from concourse.tile import TileContext

N_CORES = 8
B, S, D = 32, 2048, 64
BPC = B // N_CORES  # batches per core
NK = S // 128  # 16 k-chunks
NP = NK // 2  # 8 chunk pairs
NH = 2  # q halves
QH = S // NH  # 1024
VW = 66  # V free width: 64 d + 1 ones + 1 pad
NEG = -240.0  # additive mask value; max normal of TRN fp8e4

# which chunk pairs fold their mask on the PE (fp8 quadrant matmuls) vs
# multiplying on the DVE (bf16). Tune to balance PE vs DVE vs DMA load.
PE_MASK_PAIRS = (7,)
DVE_MASK_PAIRS = tuple(p for p in range(NP) if p not in PE_MASK_PAIRS)
NPEP = len(PE_MASK_PAIRS)
NDVP = len(DVE_MASK_PAIRS)
# pairs whose exp runs as a Schraudolph fast-exp on the DVE (one
# tensor_scalar producing bf16 bits via an int16 affine) instead of on
# ScalarE. Must be PE-masked pairs (the additive -240 fold makes masked
# entries underflow to ~2^-44 in the bit-trick too).
SCHRAUD_PAIRS = ()
# i16 = s * (0.125 * log2(e) * 128) + (16256 - 5.5); bitcast int16->bf16
SCH_A = 23.083120654223414
SCH_B = 16250.5

F32 = mybir.dt.float32
BF16 = mybir.dt.bfloat16
FP8 = mybir.dt.float8e4
EXP = mybir.ActivationFunctionType.Exp

_CACHED_NC = None


def build_nc():
    global _CACHED_NC
    if _CACHED_NC is not None:
        return _CACHED_NC
    nc = bacc.Bacc("TRN2", target_bir_lowering=False)
    QT = nc.dram_tensor("qt", [BPC, 128, S], BF16, kind="ExternalInput")
    KT = nc.dram_tensor("kt", [BPC, 128, S], BF16, kind="ExternalInput")
    V2 = nc.dram_tensor("v2", [BPC, 128, NK * VW], BF16, kind="ExternalInput")
    # additive mask for PE pairs: {0, -240} fp8, [b, h, p, pep, c01, q];
    # the c01=1 chunk is stored rolled by 64 partitions
    M8 = nc.dram_tensor(
        "m8", [BPC, NH, 128, max(NPEP, 1), 2, QH], FP8, kind="ExternalInput"
    )
    # keep-mask for DVE pairs: {1, 0} bf16, [b, h, p, dvp, c01, j, q]
    NM = nc.dram_tensor(
        "nm", [BPC, NH, 128, max(NDVP, 1), 2, 2, 512], BF16, kind="ExternalInput"
    )
    # identity duplicated into both partition halves: id2[p, c] = (p % 64 == c)
    ID2 = nc.dram_tensor("id2", [128, 64], FP8, kind="ExternalInput")
    ID66 = nc.dram_tensor("id66", [VW, VW], BF16, kind="ExternalInput")
    OUT = nc.dram_tensor("out", [BPC, NH, 8, 128, D], F32, kind="ExternalOutput")

    with TileContext(nc) as tc:
        with (
            tc.tile_pool(name="cn", bufs=1) as const_pool,
            tc.tile_pool(name="qk", bufs=2) as qk_pool,
            tc.tile_pool(name="vp", bufs=2) as v_pool,
            tc.tile_pool(name="m8", bufs=3) as m_pool,
            tc.tile_pool(name="pt", bufs=6) as pt_pool,
            tc.tile_pool(name="os", bufs=2) as ots_pool,
            tc.tile_pool(name="sc", bufs=2, space="PSUM") as sc_pool,
            tc.tile_pool(name="pv", bufs=2, space="PSUM") as pv_pool,
            tc.tile_pool(name="ou", bufs=2) as out_pool,
            tc.tile_pool(name="mi", bufs=4) as misc_pool,
        ):
            id2 = const_pool.tile([128, 64], FP8, tag="id2")
            nc.sync.dma_start(out=id2, in_=ID2[:, :])
            id66 = const_pool.tile([VW, VW], BF16, tag="id66")
            nc.sync.dma_start(out=id66, in_=ID66[:, :])
            for b in range(BPC):
                qt = qk_pool.tile([128, S], BF16, tag="qt")
                kt = qk_pool.tile([128, S], BF16, tag="kt")
                v2 = v_pool.tile([128, NK * VW], BF16, tag="v2")
                nc.sync.dma_start(out=qt, in_=QT[b])
                nc.sync.dma_start(out=kt, in_=KT[b])
                nc.sync.dma_start(out=v2, in_=V2[b])
                for h in range(NH):
                    m8 = m_pool.tile([128, max(NPEP, 1), 2, QH], FP8, tag="m8")
                    nc.sync.dma_start(out=m8, in_=M8[b, h])
                    nm = m_pool.tile([128, max(NDVP, 1), 2, 2, 512], BF16, tag="nm")
                    nc.sync.dma_start(out=nm, in_=NM[b, h])
                    # transposed PV accumulator: [d'=66, q=1024], 2 banks
                    outT = pv_pool.tile([VW, QH], F32, tag="pv")
                    for p in range(NP):
                        c0, c1 = 2 * p, 2 * p + 1
                        pe_mask = p in PE_MASK_PAIRS
                        pep = PE_MASK_PAIRS.index(p) if pe_mask else 0
                        dvp = DVE_MASK_PAIRS.index(p) if not pe_mask else 0
                        schraud = p in SCHRAUD_PAIRS
                        # one pt tile per pair [p, j, c01, q] so the DVE
                        # mask multiply is a single 2048-wide op
                        ptp = pt_pool.tile(
                            [128, 2, 2, 512],
                            mybir.dt.int16 if schraud else BF16,
                            tag="pt",
                        )
                        for j in range(2):
                            q0 = h * QH + j * 512
                            sc = sc_pool.tile([128, 2, 512], F32, tag="sc")
                            # QK on alternating 64-row halves; adjacent
                            # instructions + shared tile -> co-issue
                            nc.tensor.matmul(
                                sc[:, 0, :],
                                kt[0:64, c0 * 128 : (c0 + 1) * 128],
                                qt[0:64, q0 : q0 + 512],
                                start=True,
                                stop=not pe_mask,
                            )
                            nc.tensor.matmul(
                                sc[:, 1, :],
                                kt[64:128, c1 * 128 : (c1 + 1) * 128],
                                qt[64:128, q0 : q0 + 512],
                                start=True,
                                stop=not pe_mask,
                            )
                            if pe_mask:
                                jl = slice(j * 512, (j + 1) * 512)
                                # 4 concurrent 64x64 quadrant matmuls add the
                                # fp8 mask onto the scores in PSUM
                                nc.tensor.matmul(
                                    sc[0:64, 0, :],
                                    id2[0:64, :],
                                    m8[0:64, pep, 0, jl],
                                    start=False,
                                    stop=True,
                                )
                                nc.tensor.matmul(
                                    sc[64:128, 0, :],
                                    id2[64:128, :],
                                    m8[64:128, pep, 0, jl],
                                    start=False,
                                    stop=True,
                                )
                                nc.tensor.matmul(
                                    sc[64:128, 1, :],
                                    id2[0:64, :],
                                    m8[0:64, pep, 1, jl],
                                    start=False,
                                    stop=True,
                                )
                                nc.tensor.matmul(
                                    sc[0:64, 1, :],
                                    id2[64:128, :],
                                    m8[64:128, pep, 1, jl],
                                    start=False,
                                    stop=True,
                                )
                            if schraud:
                                nc.vector.tensor_scalar(
                                    ptp[:, j],
                                    sc,
                                    SCH_A,
                                    SCH_B,
                                    mybir.AluOpType.mult,
                                    mybir.AluOpType.add,
                                )
                            else:
                                nc.scalar.activation(ptp[:, j], sc, EXP, scale=0.125)
                        if not pe_mask:
                            # nm is [p, c01, j, q]; view as [p, j, c01, q]
                            nc.vector.tensor_mul(
                                ptp,
                                ptp,
                                nm[:, dvp].rearrange("p c j q -> p j c q"),
                            )
                        ptv = ptp.bitcast(BF16) if schraud else ptp
                        for ci, c in enumerate((c0, c1)):
                            for j in range(2):
                                jl = slice(j * 512, (j + 1) * 512)
                                nc.tensor.matmul(
                                    outT[:, jl],
                                    v2[:, c * VW : (c + 1) * VW],
                                    ptv[:, j, ci, :],
                                    start=(c == 0),
                                    stop=(c == NK - 1),
                                )
                    # epilogue: outT -> SBUF bf16 -> matmul-transpose each
                    # 128-q block -> reciprocal of row-sum col -> scale
                    ots = ots_pool.tile([VW, QH], BF16, tag="ots")
                    nc.vector.tensor_copy(ots, outT)
                    ot_all = out_pool.tile([128, 8, D], F32, tag="ot")
                    for r in range(2):
                        trans = pv_pool.tile([128, 4, 128], F32, tag="pv", name=f"tr{r}")
                        for qq in range(4):
                            qb = r * 4 + qq
                            nc.tensor.matmul(
                                trans[:, qq, 0:VW],
                                ots[:, qb * 128 : (qb + 1) * 128],
                                id66,
                                start=True,
                                stop=True,
                            )
                        rec = misc_pool.tile([128, 4], F32, tag="rec")
                        nc.vector.reciprocal(rec, trans[:, :, 64])
                        for qq in range(4):
                            nc.vector.tensor_scalar_mul(
                                ot_all[:, r * 4 + qq, :],
                                trans[:, qq, 0:64],
                                rec[:, qq : qq + 1],
                            )
                    nc.sync.dma_start(
                        out=OUT[b, h].rearrange("a p d -> p a d"), in_=ot_all
                    )
    nc.compile()
    _CACHED_NC = nc
    return nc


def prep_inputs(Q, K, V, mask):
    """Host-side layout prep (transposes, duplication for row tiling, bf16)."""
    Q = np.ascontiguousarray(np.asarray(Q, dtype=np.float32))
    K = np.ascontiguousarray(np.asarray(K, dtype=np.float32))
    V = np.ascontiguousarray(np.asarray(V, dtype=np.float32))
    mask = np.asarray(mask)
    QT1 = Q.transpose(0, 2, 1)  # [B, D, S]
    KT1 = K.transpose(0, 2, 1)
    QT = np.ascontiguousarray(
        np.concatenate([QT1, QT1], axis=1).astype(ml_dtypes.bfloat16)
    )  # [B, 128, S]
    KT = np.ascontiguousarray(
        np.concatenate([KT1, KT1], axis=1).astype(ml_dtypes.bfloat16)
    )
    # V with ones column (row-sum trick) + pad, interleaved so each SBUF
    # partition's 16 chunks are contiguous in DRAM: [B, 128, 16*VW]
    V66 = np.zeros((B, S, VW), dtype=ml_dtypes.bfloat16)
    V66[:, :, :64] = V.astype(ml_dtypes.bfloat16)
    V66[:, :, 64] = 1.0
    V2 = np.ascontiguousarray(
        V66.reshape(B, NK, 128, VW).transpose(0, 2, 1, 3).reshape(B, 128, NK * VW)
    )
    mt = mask.astype(bool).transpose(0, 2, 1)  # [B, k, q]
    mt = mt.reshape(B, NP, 2, 128, NH, QH)  # [b, pair, c01, p, h, q]
    # PE pairs: additive fp8; odd chunk rolled by 64 partitions so the
    # off-diagonal quadrant matmuls route its rows to the right partitions
    pe = mt[:, list(PE_MASK_PAIRS)]  # [b, pep, c01, p, h, q]
    pe = np.stack([pe[:, :, 0], np.roll(pe[:, :, 1], -64, axis=2)], axis=2)
    M8 = np.ascontiguousarray(
        (pe.astype(np.float32) * NEG)
        .transpose(0, 4, 3, 1, 2, 5)  # [b, h, p, pep, c01, q]
        .astype(ml_dtypes.float8_e4m3)
    )
    # DVE pairs: keep-mask bf16 [b, h, p, dvp, c01, j, q512]
    dv = ~mt[:, list(DVE_MASK_PAIRS)]  # [b, dvp, c01, p, h, q]
    dv = dv.reshape(B, NDVP, 2, 128, NH, 2, 512)
    NM = np.ascontiguousarray(
        dv.transpose(0, 4, 3, 1, 2, 5, 6).astype(ml_dtypes.bfloat16)
    )  # [b, h, p, dvp, c01, j, q]
    id2 = np.zeros((128, 64), dtype=ml_dtypes.float8_e4m3)
    id2[np.arange(128), np.arange(128) % 64] = 1.0
    id66 = np.eye(VW, dtype=ml_dtypes.bfloat16)
    return QT, KT, V2, M8, NM, id2, id66


def make_in_maps(Q, K, V, mask):
    QT, KT, V2, M8, NM, id2, id66 = prep_inputs(Q, K, V, mask)
    in_maps = []
    for i in range(N_CORES):
        sl = slice(i * BPC, (i + 1) * BPC)
        in_maps.append(
            {
                "qt": QT[sl],
                "kt": KT[sl],
                "v2": V2[sl],
                "m8": M8[sl],
                "nm": NM[sl],
                "id2": id2,
                "id66": id66,
            }
        )
    return in_maps


def kernel(Q, K, V, mask, dk, **run_kwargs):
    assert int(dk) == D
    nc = build_nc()
    in_maps = make_in_maps(Q, K, V, mask)
    res = run_bass_kernel_spmd(nc, in_maps, list(range(N_CORES)), **run_kwargs)
    out = np.concatenate(
        [res.results[i]["out"].reshape(BPC, S, D) for i in range(N_CORES)], axis=0
    )
    if run_kwargs:
        kernel.last_results = res
    return out


# revision 19
# speedup vs baseline: 1.3043x; 1.0627x over previous
"""Masked attention on 8 TRN2 NeuronCores.

Full-input contract: kernel(**inputs) takes the complete Q/K/V/mask/dk and
returns the full [32, 2048, 64] output. Internally shards batch 32 -> 4 per
core (data parallel, no communication).

Per-core kernel (4 batches of S=2048, D=64). Scores computed TRANSPOSED
(S^T[k,q] = K @ Q^T) so exp's output P^T is already in the layout the P@V
matmul consumes. v3 design notes (calibrated to this part's PE clock, which
duty-cycles ~1.28/0.68 GHz on a fixed ~102us firmware period):
  - every matmul operand is bf16/fp8 (fp32/fp32r matmuls leave the clock
    governor cold)
  - QK^T: bf16, 2-way PE row tiling; a chunk PAIR shares one PSUM tile
    [128, 2, 512] so the two row-tiled matmuls become adjacent instructions
    with identical dependencies and can co-issue on opposite 64-row halves
  - mask: split between engines to balance load.
      PE pairs (PE_MASK_PAIRS): additive fp8 {0,-240} mask accumulated onto
        scores by 4 concurrent 64x64 quadrant identity-matmuls (odd chunk
        stored half-rolled so off-diagonal quadrants route its rows); exp
        then underflows masked entries to ~1e-13.
      DVE pairs (the rest): bf16 {1,0} keep-mask multiplied into exp's
        output (2x-mode tensor_tensor, one op per pair-tile).
  - exp on ScalarE (scale=1/8 folded in), bf16 out, one op per [128,2,512]
    pair-tile
  - P@V transposed: V chunk (with a ones column computing the softmax row
    sum) is the stationary operand, P^T the moving one -> outT[66, q]
    accumulates in PSUM over k-chunks
  - epilogue: copy outT to SBUF (bf16), transpose each 128-q block back via
    a regular matmul against a 66-wide identity, then DVE reciprocal of the
    row-sum column + per-partition scale -> natural [q, d] fp32 output
"""

import sys

import numpy as np

for _p in ("/opt/trn_rl_repo", "/root/.axon_site/_ro/trn_rl_repo"):
    if _p not in sys.path:
        sys.path.append(_p)

import ml_dtypes

import concourse.bacc as bacc
import concourse.bass as bass
import concourse.mybir as mybir
from concourse.bass_utils import run_bass_kernel_spmd
from concourse.tile import TileContext

N_CORES = 8
B, S, D = 32, 2048, 64
BPC = B // N_CORES  # batches per core
NK = S // 128  # 16 k-chunks
NP = NK // 2  # 8 chunk pairs
NH = 2  # q halves
QH = S // NH  # 1024
VW = 66  # V free width: 64 d + 1 ones + 1 pad
NEG = -240.0  # additive mask value; max normal of TRN fp8e4

# which chunk pairs fold their mask on the PE (fp8 quadrant matmuls) vs
# multiplying on the DVE (bf16). Tune to balance PE vs DVE vs DMA load.
PE_MASK_PAIRS = (7,)
DVE_MASK_PAIRS = tuple(p for p in range(NP) if p not in PE_MASK_PAIRS)
NPEP = len(PE_MASK_PAIRS)
NDVP = len(DVE_MASK_PAIRS)
# pairs whose exp runs as a Schraudolph fast-exp on the DVE (one
# tensor_scalar producing bf16 bits via an int16 affine) instead of on
# ScalarE. Must be PE-masked pairs (the additive -240 fold makes masked
# entries underflow to ~2^-44 in the bit-trick too).
SCHRAUD_PAIRS = ()
# i16 = s * (0.125 * log2(e) * 128) + (16256 - 5.5); bitcast int16->bf16
SCH_A = 23.083120654223414
SCH_B = 16250.5

F32 = mybir.dt.float32
BF16 = mybir.dt.bfloat16
FP8 = mybir.dt.float8e4
EXP = mybir.ActivationFunctionType.Exp

_CACHED_NC = None


def build_nc():
    global _CACHED_NC
    if _CACHED_NC is not None:
        return _CACHED_NC
    nc = bacc.Bacc("TRN2", target_bir_lowering=False)
    QT = nc.dram_tensor("qt", [BPC, 128, S], BF16, kind="ExternalInput")
    KT = nc.dram_tensor("kt", [BPC, 128, S], BF16, kind="ExternalInput")
    V2 = nc.dram_tensor("v2", [BPC, 128, NK * VW], BF16, kind="ExternalInput")
    # additive mask for PE pairs: {0, -240} fp8, [b, h, p, pep, c01, q];
    # the c01=1 chunk is stored rolled by 64 partitions
    M8 = nc.dram_tensor(
        "m8", [BPC, NH, 128, max(NPEP, 1), 2, QH], FP8, kind="ExternalInput"
    )
    # keep-mask for DVE pairs: {1, 0} bf16, [b, h, p, dvp, c01, j, q]
    NM = nc.dram_tensor(
        "nm", [BPC, NH, 128, max(NDVP, 1), 2, 2, 512], BF16, kind="ExternalInput"
    )
    # identity duplicated into both partition halves: id2[p, c] = (p % 64 == c)
    ID2 = nc.dram_tensor("id2", [128, 64], FP8, kind="ExternalInput")
    ID66 = nc.dram_tensor("id66", [VW, VW], BF16, kind="ExternalInput")
    OUT = nc.dram_tensor("out", [BPC, NH, 8, 128, D], F32, kind="ExternalOutput")

    with TileContext(nc) as tc:
        with (
            tc.tile_pool(name="cn", bufs=1) as const_pool,
            tc.tile_pool(name="qk", bufs=2) as qk_pool,
            tc.tile_pool(name="vp", bufs=2) as v_pool,
            tc.tile_pool(name="m8", bufs=3) as m_pool,
            tc.tile_pool(name="pt", bufs=6) as pt_pool,
            tc.tile_pool(name="os", bufs=2) as ots_pool,
            tc.tile_pool(name="sc", bufs=2, space="PSUM") as sc_pool,
            tc.tile_pool(name="pv", bufs=2, space="PSUM") as pv_pool,
            tc.tile_pool(name="ou", bufs=2) as out_pool,
            tc.tile_pool(name="mi", bufs=4) as misc_pool,
        ):
            id2 = const_pool.tile([128, 64], FP8, tag="id2")
            nc.sync.dma_start(out=id2, in_=ID2[:, :])
            id66 = const_pool.tile([VW, VW], BF16, tag="id66")
            nc.sync.dma_start(out=id66, in_=ID66[:, :])
            for b in range(BPC):
                qt = qk_pool.tile([128, S], BF16, tag="qt")
                kt = qk_pool.tile([128, S], BF16, tag="kt")
                v2 = v_pool.tile([128, NK * VW], BF16, tag="v2")
                nc.sync.dma_start(out=qt, in_=QT[b])
                nc.sync.dma_start(out=kt, in_=KT[b])
                nc.sync.dma_start(out=v2, in_=V2[b])
                for h in range(NH):
                    m8 = m_pool.tile([128, max(NPEP, 1), 2, QH], FP8, tag="m8")
                    nc.sync.dma_start(out=m8, in_=M8[b, h])
                    nm = m_pool.tile([128, max(NDVP, 1), 2, 2, 512], BF16, tag="nm")
                    nc.sync.dma_start(out=nm, in_=NM[b, h])
                    # transposed PV accumulator: [d'=66, q=1024], 2 banks
                    outT = pv_pool.tile([VW, QH], F32, tag="pv")
                    for p in range(NP):
                        c0, c1 = 2 * p, 2 * p + 1
                        pe_mask = p in PE_MASK_PAIRS
                        pep = PE_MASK_PAIRS.index(p) if pe_mask else 0
                        dvp = DVE_MASK_PAIRS.index(p) if not pe_mask else 0
                        schraud = p in SCHRAUD_PAIRS
                        # one pt tile per pair [p, j, c01, q] so the DVE
                        # mask multiply is a single 2048-wide op
                        ptp = pt_pool.tile(
                            [128, 2, 2, 512],
                            mybir.dt.int16 if schraud else BF16,
                            tag="pt",
                        )
                        for j in range(2):
                            q0 = h * QH + j * 512
                            sc = sc_pool.tile([128, 2, 512], F32, tag="sc")
                            # QK on alternating 64-row halves; adjacent
                            # instructions + shared tile -> co-issue
                            nc.tensor.matmul(
                                sc[:, 0, :],
                                kt[0:64, c0 * 128 : (c0 + 1) * 128],
                                qt[0:64, q0 : q0 + 512],
                                start=True,
                                stop=not pe_mask,
                            )
                            nc.tensor.matmul(
                                sc[:, 1, :],
                                kt[64:128, c1 * 128 : (c1 + 1) * 128],
                                qt[64:128, q0 : q0 + 512],
                                start=True,
                                stop=not pe_mask,
                            )
                            if pe_mask:
                                jl = slice(j * 512, (j + 1) * 512)
                                # 4 concurrent 64x64 quadrant matmuls add the
                                # fp8 mask onto the scores in PSUM
                                nc.tensor.matmul(
                                    sc[0:64, 0, :],
                                    id2[0:64, :],
                                    m8[0:64, pep, 0, jl],
                                    start=False,
                                    stop=True,
                                )
                                nc.tensor.matmul(
                                    sc[64:128, 0, :],
                                    id2[64:128, :],
                                    m8[64:128, pep, 0, jl],
                                    start=False,
                                    stop=True,
                                )
                                nc.tensor.matmul(
                                    sc[64:128, 1, :],
                                    id2[0:64, :],
                                    m8[0:64, pep, 1, jl],
                                    start=False,
                                    stop=True,
                                )
                                nc.tensor.matmul(
                                    sc[0:64, 1, :],
                                    id2[64:128, :],
                                    m8[64:128, pep, 1, jl],
                                    start=False,
                                    stop=True,
                                )
                            if schraud:
                                nc.vector.tensor_scalar(
                                    ptp[:, j],
                                    sc,
                                    SCH_A,
                                    SCH_B,
                                    mybir.AluOpType.mult,
                                    mybir.AluOpType.add,
                                )
                            else:
                                nc.scalar.activation(ptp[:, j], sc, EXP, scale=0.125)
                            if not pe_mask:
                                # nm is [p, c01, j, q]: slice j, keep [p, c, q]
                                nc.vector.tensor_mul(
                                    ptp[:, j], ptp[:, j], nm[:, dvp, :, j, :]
                                )
                        ptv = ptp.bitcast(BF16) if schraud else ptp
                        for ci, c in enumerate((c0, c1)):
                            for j in range(2):
                                jl = slice(j * 512, (j + 1) * 512)
                                nc.tensor.matmul(
                                    outT[:, jl],
                                    v2[:, c * VW : (c + 1) * VW],
                                    ptv[:, j, ci, :],
                                    start=(c == 0),
                                    stop=(c == NK - 1),
                                )
                    # epilogue: outT -> SBUF bf16 -> matmul-transpose each
                    # 128-q block -> reciprocal of row-sum col -> scale
                    ots = ots_pool.tile([VW, QH], BF16, tag="ots")
                    nc.vector.tensor_copy(ots, outT)
                    ot_all = out_pool.tile([128, 8, D], F32, tag="ot")
                    for r in range(2):
                        trans = pv_pool.tile([128, 4, 128], F32, tag="pv", name=f"tr{r}")
                        for qq in range(4):
                            qb = r * 4 + qq
                            nc.tensor.matmul(
                                trans[:, qq, 0:VW],
                                ots[:, qb * 128 : (qb + 1) * 128],
                                id66,
                                start=True,
                                stop=True,
                            )
                        rec = misc_pool.tile([128, 4], F32, tag="rec")
                        nc.vector.reciprocal(rec, trans[:, :, 64])
                        for qq in range(4):
                            nc.vector.tensor_scalar_mul(
                                ot_all[:, r * 4 + qq, :],
                                trans[:, qq, 0:64],
                                rec[:, qq : qq + 1],
                            )
                    nc.sync.dma_start(
                        out=OUT[b, h].rearrange("a p d -> p a d"), in_=ot_all
                    )
    nc.compile()
    _CACHED_NC = nc
    return nc


def prep_inputs(Q, K, V, mask):
    """Host-side layout prep (transposes, duplication for row tiling, bf16)."""
    Q = np.ascontiguousarray(np.asarray(Q, dtype=np.float32))
    K = np.ascontiguousarray(np.asarray(K, dtype=np.float32))
    V = np.ascontiguousarray(np.asarray(V, dtype=np.float32))
    mask = np.asarray(mask)
    QT1 = Q.transpose(0, 2, 1)  # [B, D, S]
    KT1 = K.transpose(0, 2, 1)
    QT = np.ascontiguousarray(
        np.concatenate([QT1, QT1], axis=1).astype(ml_dtypes.bfloat16)
    )  # [B, 128, S]
    KT = np.ascontiguousarray(
        np.concatenate([KT1, KT1], axis=1).astype(ml_dtypes.bfloat16)
    )
    # V with ones column (row-sum trick) + pad, interleaved so each SBUF
    # partition's 16 chunks are contiguous in DRAM: [B, 128, 16*VW]
    V66 = np.zeros((B, S, VW), dtype=ml_dtypes.bfloat16)
    V66[:, :, :64] = V.astype(ml_dtypes.bfloat16)
    V66[:, :, 64] = 1.0
    V2 = np.ascontiguousarray(
        V66.reshape(B, NK, 128, VW).transpose(0, 2, 1, 3).reshape(B, 128, NK * VW)
    )
    mt = mask.astype(bool).transpose(0, 2, 1)  # [B, k, q]
    mt = mt.reshape(B, NP, 2, 128, NH, QH)  # [b, pair, c01, p, h, q]
    # PE pairs: additive fp8; odd chunk rolled by 64 partitions so the
    # off-diagonal quadrant matmuls route its rows to the right partitions
    pe = mt[:, list(PE_MASK_PAIRS)]  # [b, pep, c01, p, h, q]
    pe = np.stack([pe[:, :, 0], np.roll(pe[:, :, 1], -64, axis=2)], axis=2)
    M8 = np.ascontiguousarray(
        (pe.astype(np.float32) * NEG)
        .transpose(0, 4, 3, 1, 2, 5)  # [b, h, p, pep, c01, q]
        .astype(ml_dtypes.float8_e4m3)
    )
    # DVE pairs: keep-mask bf16 [b, h, p, dvp, c01, j, q512]
    dv = ~mt[:, list(DVE_MASK_PAIRS)]  # [b, dvp, c01, p, h, q]
    dv = dv.reshape(B, NDVP, 2, 128, NH, 2, 512)
    NM = np.ascontiguousarray(
        dv.transpose(0, 4, 3, 1, 2, 5, 6).astype(ml_dtypes.bfloat16)
    )  # [b, h, p, dvp, c01, j, q]
    id2 = np.zeros((128, 64), dtype=ml_dtypes.float8_e4m3)
    id2[np.arange(128), np.arange(128) % 64] = 1.0
    id66 = np.eye(VW, dtype=ml_dtypes.bfloat16)
    return QT, KT, V2, M8, NM, id2, id66


def make_in_maps(Q, K, V, mask):
    QT, KT, V2, M8, NM, id2, id66 = prep_inputs(Q, K, V, mask)
    in_maps = []
    for i in range(N_CORES):
        sl = slice(i * BPC, (i + 1) * BPC)
        in_maps.append(
            {
                "qt": QT[sl],
                "kt": KT[sl],
                "v2": V2[sl],
                "m8": M8[sl],
                "nm": NM[sl],
                "id2": id2,
                "id66": id66,
            }
        )
    return in_maps


def kernel(Q, K, V, mask, dk, **run_kwargs):
    assert int(dk) == D
    nc = build_nc()
    in_maps = make_in_maps(Q, K, V, mask)
    res = run_bass_kernel_spmd(nc, in_maps, list(range(N_CORES)), **run_kwargs)
    out = np.concatenate(
        [res.results[i]["out"].reshape(BPC, S, D) for i in range(N_CORES)], axis=0
    )
    if run_kwargs:
        kernel.last_results = res
    return out
